# revision 14
# baseline (speedup 1.0000x reference)
"""Trainium2 Bass kernel for nn_CrossModal_NN: data-parallel over batch (8 cores).

Per core (128 batch rows):
  encoders -> multiscale -> 2x MMFA (attention + 64MB bilinear) -> heads.
The bilinear einsum('bsi,oij,bsj->bso') dominates: computed as a PE matmul
u[r,(o,j)] = sum_i c_vaT[i,r] * Wt[i,(o,j)] accumulated in PSUM, followed by a
fused multiply-reduce against c_av on DVE (with a partial ACT copy-to-bf16
offload path to balance engine load).

Self-contained: hardcodes shapes; host side preps transposed/bf16 weight
layouts and shards the batch.
"""
import numpy as np
import ml_dtypes

import concourse.bass as bass
import concourse.tile as tile
from concourse import bacc, mybir
from concourse.bass_utils import run_bass_kernel_spmd
from concourse.masks import make_identity

F32 = mybir.dt.float32
BF16 = mybir.dt.bfloat16
AF = mybir.ActivationFunctionType
OP = mybir.AluOpType

NCORES = 8
B = 1024
BL = B // NCORES          # 128 rows per core
S, D, ENC = 3, 256, 256 * 4  # ENC=1024
IMG, AUD = 4096, 1024
SD = S * D                # 768
OUT1, LAT, NCLS = 512, 128, 10
NORM = 1.0 / np.sqrt(256.0)

# fraction knob: which (o-pair, s) banks take the ACT-copy offload path
# (o_i*3+s) % 3 in OFFLOAD_RESIDUES -> offload
OFFLOAD_MOD = 3
OFFLOAD_RESIDUES = (1, 2)

_prog_cache = {}


def _blk(t, c, w=128):
    """column block c of width w from a [128, n*w] tile"""
    return t[:, c * w:(c + 1) * w]


def build_program():
    import os
    stage = int(os.environ.get("KSTAGE", "9"))
    sub = int(os.environ.get("KSUB", "9"))
    nc = bacc.Bacc("TRN2", target_bir_lowering=False)

    def din(name, shape, dt=BF16):
        return nc.dram_tensor(name, shape, dt, kind="ExternalInput").ap()

    def dout(name, shape):
        return nc.dram_tensor(name, shape, F32, kind="ExternalOutput").ap()

    imgT_d = din("imgT", [IMG, BL])
    audT_d = din("audT", [AUD, BL])
    visWT_d = din("visWT", [IMG, ENC])
    audWT_d = din("audWT", [AUD, ENC])
    msvWT_d = din("msvWT", [ENC, SD])
    msaWT_d = din("msaWT", [ENC, SD])
    mmfaWT_d = din("mmfaWT", [D, 8 * D])
    bilWT_d = din("bilWT", [D, D * D])
    outW1T_d = din("outW1T", [SD, OUT1])
    outW2T_d = din("outW2T", [OUT1, LAT])
    cvW1T_d = din("cvW1T", [LAT, 32])
    cvW2T_d = din("cvW2T", [32, NCLS])
    caW1T_d = din("caW1T", [LAT, 32])
    caW2T_d = din("caW2T", [32, NCLS])
    visb_d = din("visb_rep", [128, ENC], F32)
    audb_d = din("audb_rep", [128, ENC], F32)
    msvb_d = din("msvb_rep", [128, SD], F32)
    msab_d = din("msab_rep", [128, SD], F32)
    qkb_d = din("qkb_rep", [128, 4 * D], F32)
    cb_d = din("cb_cols", [128, 4], F32)      # col c*2+kc: (b4+b5) / (b6+b7) chunks
    outb1_d = din("outb1T", [128, 4], F32)
    outb2_d = din("outb2T", [128, 1], F32)
    tc_d = din("tc_rep", [128, 1], F32)

    fv_d = dout("fv", [BL, LAT])
    fa_d = dout("fa", [BL, LAT])
    pv_d = dout("pv", [BL, NCLS])
    pa_d = dout("pa", [BL, NCLS])

    with tile.TileContext(nc) as tc:
        with (
            tc.tile_pool(name="const", bufs=1) as const,
            tc.tile_pool(name="res", bufs=1) as res,
            tc.tile_pool(name="acts", bufs=1) as acts,
            tc.tile_pool(name="encw", bufs=3) as encw,
            tc.tile_pool(name="wbil", bufs=6) as wbil,
            tc.tile_pool(name="scr", bufs=3) as scr,
            tc.tile_pool(name="gps", bufs=2, space="PSUM") as gps,
            tc.tile_pool(name="ups", bufs=6, space="PSUM") as ups,
        ):
            identf = const.tile([128, 128], F32, tag="identf")
            make_identity(nc, identf[:])
            identb = const.tile([128, 128], BF16, tag="identb")
            make_identity(nc, identb[:])

            # ---- resident loads ----
            def load(pool, name, dram, shape, dt=BF16, tag=None):
                t = pool.tile(shape, dt, tag=tag or name)
                nc.sync.dma_start(t[:], dram[:, :])
                return t

            # imgT/audT as [128, nblk*128] lhsT block tiles
            def load_rowchunks(pool, name, dram, nblk, ncols):
                """dram [nblk*128, ncols] -> SBUF [128, nblk*ncols] col-blocks"""
                t = pool.tile([128, nblk * ncols], BF16, tag=name)
                nc.sync.dma_start(
                    t[:].rearrange("p (c b) -> p c b", b=ncols),
                    dram.rearrange("(c p) b -> p c b", p=128))
                return t

            imgT = load_rowchunks(res, "imgT", imgT_d, IMG // 128, BL)
            audT = load_rowchunks(res, "audT", audT_d, AUD // 128, BL)

            mmfaWT = [res.tile([128, 8 * D], BF16, tag=f"mmfaWT{i}",
                               name=f"mmfaWT{i}")
                      for i in range(2)]
            for i in range(2):
                nc.sync.dma_start(mmfaWT[i][:],
                                  mmfaWT_d[i * 128:(i + 1) * 128, :])
            # outW1T blocks: kc -> rows chunk kc (full 512 cols)
            outW1T = load_rowchunks(res, "outW1T", outW1T_d, 6, OUT1)
            outW2T = load_rowchunks(res, "outW2T", outW2T_d, 4, LAT)
            cvW1T = load(res, "cvW1T", cvW1T_d, [LAT, 32])
            caW1T = load(res, "caW1T", caW1T_d, [LAT, 32])
            cvW2T = load(res, "cvW2T", cvW2T_d, [32, NCLS])
            caW2T = load(res, "caW2T", caW2T_d, [32, NCLS])
            visb = load(res, "visb", visb_d, [128, ENC], F32)
            audb = load(res, "audb", audb_d, [128, ENC], F32)
            msvb = load(res, "msvb", msvb_d, [128, SD], F32)
            msab = load(res, "msab", msab_d, [128, SD], F32)
            qkb = load(res, "qkb", qkb_d, [128, 4 * D], F32)
            cbc = load(res, "cbc", cb_d, [128, 4], F32)
            outb1 = load(res, "outb1", outb1_d, [128, 4], F32)
            outb2 = load(res, "outb2", outb2_d, [128, 1], F32)
            tcr = load(res, "tcr", tc_d, [128, 1], F32)

            def transpose_blk(dst_slice, src_slice, dt=BF16):
                """PE-transpose a [128,128] block; ACT copyback."""
                ps = gps.tile([128, 128], dt, tag="gps")
                ident = identb if dt == BF16 else identf
                nc.tensor.transpose(ps[:], src_slice, ident[:])
                nc.scalar.activation(dst_slice, ps[:], AF.Copy)

            # ---- encoders: x_nat = relu(x @ W.T + b) ----
            def emit_encoder(xT, WT_d, nK, bias, tag):
                x_nat = acts.tile([128, ENC], BF16, tag=f"{tag}_nat")
                ps = [gps.tile([128, 512], F32, tag="gps", name=f"encps{h}")
                      for h in range(2)]
                for dc in range(nK):
                    w = encw.tile([128, ENC], BF16, tag="encw")
                    nc.sync.dma_start(w[:], WT_d[dc * 128:(dc + 1) * 128, :])
                    for h in range(2):
                        nc.tensor.matmul(ps[h][:], _blk(xT, dc),
                                         w[:, h * 512:(h + 1) * 512],
                                         start=(dc == 0), stop=(dc == nK - 1))
                for h in range(2):
                    tmp = scr.tile([128, 512], F32, tag="enc_tmp")
                    nc.vector.tensor_add(tmp[:], ps[h][:],
                                         bias[:, h * 512:(h + 1) * 512])
                    nc.scalar.activation(x_nat[:, h * 512:(h + 1) * 512],
                                         tmp[:], AF.Relu)
                # transpose to T-layout blocks
                xT_out = acts.tile([128, ENC], BF16, tag=f"{tag}_T")
                for c in range(ENC // 128):
                    transpose_blk(_blk(xT_out, c), _blk(x_nat, c))
                return x_nat, xT_out

            _, visT = emit_encoder(imgT, visWT_d, IMG // 128, visb, "vis")
            _, audTl = emit_encoder(audT, audWT_d, AUD // 128, audb, "aud")

            # ---- multiscale: v_ms = relu(vis @ msv_W.T) -> nat + T ----
            def emit_ms(xT, WT_d, bias, tag):
                x_nat = acts.tile([128, SD], BF16, tag=f"{tag}_nat")
                ps0 = gps.tile([128, 512], F32, tag="gps")
                ps1 = gps.tile([128, 256], F32, tag="gps")
                for dc in range(ENC // 128):
                    w = encw.tile([128, SD], BF16, tag="encw")
                    nc.sync.dma_start(w[:], WT_d[dc * 128:(dc + 1) * 128, :])
                    nc.tensor.matmul(ps0[:], _blk(xT, dc), w[:, 0:512],
                                     start=(dc == 0), stop=(dc == 7))
                    nc.tensor.matmul(ps1[:], _blk(xT, dc), w[:, 512:768],
                                     start=(dc == 0), stop=(dc == 7))
                tmp = scr.tile([128, 512], F32, tag="enc_tmp")
                nc.vector.tensor_add(tmp[:], ps0[:], bias[:, 0:512])
                nc.scalar.activation(x_nat[:, 0:512], tmp[:], AF.Relu)
                tmp2 = scr.tile([128, 512], F32, tag="enc_tmp")
                nc.vector.tensor_add(tmp2[:, 0:256], ps1[:], bias[:, 512:768])
                nc.scalar.activation(x_nat[:, 512:768], tmp2[:, 0:256],
                                     AF.Relu)
                xT_out = acts.tile([128, SD], BF16, tag=f"{tag}_T")
                for c in range(SD // 128):
                    transpose_blk(_blk(xT_out, c), _blk(x_nat, c))
                return x_nat, xT_out

            vms_nat, vmsT = emit_ms(visT, msvWT_d, msvb, "vms")
            ams_nat, amsT = emit_ms(audTl, msaWT_d, msab, "ams")

            # ---- MMFA ----
            def emit_mmfa(cid, xvT, xaT, xv_nat, xa_nat):
                # q/k lins (natural layout [b, (s,k)])
                qk = []
                for li, src in enumerate([xvT, xaT, xvT, xaT]):
                    qn = acts.tile([128, SD], BF16, tag=f"qk{li}_{cid}")
                    for s in range(S):
                        ps = gps.tile([128, D], F32, tag="gps")
                        for dc in range(2):
                            nc.tensor.matmul(
                                ps[:], _blk(src, s * 2 + dc),
                                mmfaWT[dc][:, li * D:(li + 1) * D],
                                start=(dc == 0), stop=(dc == 1))
                        nc.vector.tensor_add(qn[:, s * D:(s + 1) * D], ps[:],
                                             qkb[:, li * D:(li + 1) * D])
                    qk.append(qn)
                qv, qa, kv, ka = qk

                if sub < 2:
                    return None, None
                # logits[b, ai*9+s*3+t] = q_s . k_t
                logits = acts.tile([128, 36], F32, tag=f"lg{cid}")
                tscr = scr.tile([128, D], BF16, tag="tscr")
                for ai, (q, k) in enumerate([(qv, ka), (qv, kv),
                                             (qa, kv), (qa, ka)]):
                    for s in range(S):
                        for t in range(S):
                            nc.vector.scalar_tensor_tensor(
                                tscr[:], q[:, s * D:(s + 1) * D], 0.0,
                                k[:, t * D:(t + 1) * D], OP.add, OP.mult,
                                accum_out=logits[:, ai * 9 + s * 3 + t:
                                                 ai * 9 + s * 3 + t + 1])
                if sub < 3:
                    return None, None
                # softmax over t (last dim of [12, 3]) then * NORM
                expd = acts.tile([128, 36], F32, tag=f"ex{cid}")
                nc.scalar.activation(expd[:], logits[:], AF.Exp)
                sums = acts.tile([128, 12], F32, tag=f"sm{cid}")
                nc.vector.tensor_reduce(
                    sums[:], expd[:].rearrange("p (g t) -> p g t", t=3),
                    axis=mybir.AxisListType.X, op=OP.add)
                rec = acts.tile([128, 12], F32, tag=f"rc{cid}")
                nc.vector.reciprocal(rec[:], sums[:])
                attw = acts.tile([128, 36], F32, tag=f"aw{cid}")
                for g in range(12):
                    nc.vector.tensor_scalar(
                        attw[:, g * 3:(g + 1) * 3], expd[:, g * 3:(g + 1) * 3],
                        rec[:, g:g + 1], NORM, OP.mult, OP.mult)

                if sub < 4:
                    return None, None
                # alphas (natural, on gpsimd): al[ai] = sum_t attw * xsrc_t
                al_nat = []
                for ai, xsrc in enumerate([xa_nat, xv_nat, xv_nat, xa_nat]):
                    al = acts.tile([128, SD], BF16, tag=f"al{ai}_{cid}")
                    for s in range(S):
                        a0 = scr.tile([128, D], F32, tag="alacc")
                        col = lambda t: attw[:, ai * 9 + s * 3 + t:
                                             ai * 9 + s * 3 + t + 1]
                        nc.vector.tensor_scalar(
                            a0[:], xsrc[:, 0:D], col(0), None, OP.mult)
                        a1 = scr.tile([128, D], F32, tag="alacc")
                        nc.vector.scalar_tensor_tensor(
                            a1[:], xsrc[:, D:2 * D], col(1), a0[:],
                            OP.mult, OP.add)
                        nc.vector.scalar_tensor_tensor(
                            al[:, s * D:(s + 1) * D], xsrc[:, 2 * D:3 * D],
                            col(2), a1[:], OP.mult, OP.add)
                    al_nat.append(al)

                if sub < 5:
                    return None, None
                # transpose alphas to T-layout
                alT = []
                for ai in range(4):
                    t = acts.tile([128, SD], BF16, tag=f"alT{ai}_{cid}")
                    for c in range(SD // 128):
                        transpose_blk(_blk(t, c), _blk(al_nat[ai], c))
                    alT.append(t)

                if sub < 6:
                    return None, None
                # c_va = sig((lin4(al_va)+lin5(al_v)+b45) * al_v) ; c_av likewise
                cT = []
                for ci, (lA, lB, aA, aB, mT) in enumerate(
                        [(4, 5, alT[0], alT[1], alT[1]),
                         (6, 7, alT[2], alT[3], alT[3])]):
                    ct = acts.tile([128, SD], BF16, tag=f"c{ci}_{cid}")
                    for s in range(S):
                        for kc in range(2):
                            ps = gps.tile([128, 128], F32, tag="gps")
                            mms = [(lA, aA, 0), (lA, aA, 1),
                                   (lB, aB, 0), (lB, aB, 1)]
                            for mi, (li, at, dc) in enumerate(mms):
                                nc.tensor.matmul(
                                    ps[:],
                                    mmfaWT[dc][:, li * D + kc * 128:
                                               li * D + (kc + 1) * 128],
                                    _blk(at, s * 2 + dc),
                                    start=(mi == 0), stop=(mi == 3))
                            tmp = scr.tile([128, 128], F32, tag="cstt")
                            nc.vector.scalar_tensor_tensor(
                                tmp[:], ps[:], cbc[:, ci * 2 + kc:
                                                   ci * 2 + kc + 1],
                                _blk(mT, s * 2 + kc), OP.add, OP.mult)
                            nc.scalar.activation(_blk(ct, s * 2 + kc), tmp[:],
                                                 AF.Sigmoid)
                    cT.append(ct)
                cvaT, cavT = cT

                if sub < 7:
                    return None, None
                # c_av natural for the TTR in1
                cav_nat = acts.tile([128, SD], BF16, tag=f"cavn_{cid}")
                for c in range(SD // 128):
                    transpose_blk(_blk(cav_nat, c), _blk(cavT, c))

                # bilinear
                jpre = acts.tile([128, SD], F32, tag=f"jp{cid}")
                if stage < 4:
                    nc.vector.memset(jpre[:], 0.0)
                ttrs = scr.tile([128, D], BF16, tag="tscr")
                for o_i in range(128 if stage >= 4 else 0):
                    w0 = wbil.tile([128, 512], BF16, tag="wbil")
                    w1 = wbil.tile([128, 512], BF16, tag="wbil")
                    nc.sync.dma_start(
                        w0[:], bilWT_d[0:128, o_i * 512:(o_i + 1) * 512])
                    nc.sync.dma_start(
                        w1[:], bilWT_d[128:256, o_i * 512:(o_i + 1) * 512])
                    for s in range(S):
                        u = ups.tile([128, 512], F32, tag="u")
                        nc.tensor.matmul(u[:], _blk(cvaT, s * 2), w0[:],
                                         start=True, stop=False)
                        nc.tensor.matmul(u[:], _blk(cvaT, s * 2 + 1), w1[:],
                                         start=False, stop=True)
                        if (o_i * 3 + s) % OFFLOAD_MOD in OFFLOAD_RESIDUES:
                            ub = scr.tile([128, 512], BF16, tag="ubil")
                            nc.scalar.activation(ub[:], u[:], AF.Copy)
                            src0 = ub
                        else:
                            src0 = u
                        for oo in range(2):
                            o = o_i * 2 + oo
                            nc.vector.scalar_tensor_tensor(
                                ttrs[:], src0[:, oo * D:(oo + 1) * D], 0.0,
                                cav_nat[:, s * D:(s + 1) * D],
                                OP.add, OP.mult,
                                accum_out=jpre[:, s * D + o:s * D + o + 1])

                if sub < 8:
                    return None, None
                jn = acts.tile([128, SD], F32, tag=f"j{cid}")
                nc.scalar.activation(jn[:], jpre[:], AF.Sigmoid)

                # M = tc*j*xv + (1-j)*xa = j*(tc*xv - xa) + xa  (on gpsimd)
                dd = acts.tile([128, SD], F32, tag=f"dd{cid}")
                nc.vector.scalar_tensor_tensor(dd[:], xv_nat[:], tcr[:, 0:1],
                                               xa_nat[:], OP.mult,
                                               OP.subtract)
                jd = acts.tile([128, SD], F32, tag=f"jd{cid}")
                nc.gpsimd.tensor_mul(jd[:], jn[:], dd[:])
                M_nat = acts.tile([128, SD], BF16, tag=f"Mn{cid}")
                nc.gpsimd.tensor_add(M_nat[:], jd[:], xa_nat[:])
                M_T = acts.tile([128, SD], BF16, tag=f"MT{cid}")
                for c in range(SD // 128):
                    transpose_blk(_blk(M_T, c), _blk(M_nat, c))
                return M_T, M_nat

            if stage >= 3:
                M1T, M1n = emit_mmfa(0, vmsT, amsT, vms_nat, ams_nat)

            if stage >= 5:
                xv2T = acts.tile([128, SD], BF16, tag="xv2T")
                nc.gpsimd.tensor_mul(xv2T[:], M1T[:], vmsT[:])
                xa2T = acts.tile([128, SD], BF16, tag="xa2T")
                nc.gpsimd.tensor_mul(xa2T[:], M1T[:], amsT[:])
                xv2n = acts.tile([128, SD], BF16, tag="xv2n")
                nc.gpsimd.tensor_mul(xv2n[:], M1n[:], vms_nat[:])
                xa2n = acts.tile([128, SD], BF16, tag="xa2n")
                nc.gpsimd.tensor_mul(xa2n[:], M1n[:], ams_nat[:])

                M2T, _ = emit_mmfa(1, xv2T, xa2T, xv2n, xa2n)

                ovT = acts.tile([128, SD], BF16, tag="ovT")
                nc.gpsimd.tensor_mul(ovT[:], M2T[:], vmsT[:])
                oaT = acts.tile([128, SD], BF16, tag="oaT")
                nc.gpsimd.tensor_mul(oaT[:], M2T[:], amsT[:])

            # ---- heads ----
            def emit_head(oT, cW1, cW2, f_d, p_d, tag):
                h1 = acts.tile([128, OUT1], BF16, tag=f"h1{tag}")
                for m in range(4):
                    ps = gps.tile([128, 128], F32, tag="gps")
                    for kc in range(6):
                        nc.tensor.matmul(
                            ps[:],
                            outW1T[:, kc * OUT1 + m * 128:
                                   kc * OUT1 + (m + 1) * 128],
                            _blk(oT, kc),
                            start=(kc == 0), stop=(kc == 5))
                    nc.scalar.activation(_blk(h1, m), ps[:], AF.Relu,
                                         bias=outb1[:, m:m + 1])
                psf = gps.tile([128, 128], F32, tag="gps")
                for kc in range(4):
                    nc.tensor.matmul(psf[:], _blk(outW2T, kc), _blk(h1, kc),
                                     start=(kc == 0), stop=(kc == 3))
                fT = acts.tile([128, 128], F32, tag=f"fT{tag}")
                nc.scalar.activation(fT[:], psf[:], AF.Identity,
                                     bias=outb2[:, 0:1])
                fTb = acts.tile([128, 128], BF16, tag=f"fTb{tag}")
                nc.vector.tensor_copy(fTb[:], fT[:])
                # cls head
                ps2 = gps.tile([32, 128], F32, tag="gps")
                nc.tensor.matmul(ps2[:], cW1[:, :], fTb[:],
                                 start=True, stop=True)
                h2 = acts.tile([32, 128], BF16, tag=f"h2{tag}")
                nc.scalar.activation(h2[:], ps2[:], AF.Relu)
                ps3 = gps.tile([NCLS, 128], F32, tag="gps")
                nc.tensor.matmul(ps3[:], cW2[:, :], h2[:],
                                 start=True, stop=True)
                pT = acts.tile([NCLS, 128], F32, tag=f"pT{tag}")
                nc.scalar.activation(pT[:], ps3[:], AF.Copy)
                # transpose to natural + store
                fps = gps.tile([128, 128], F32, tag="gps")
                nc.tensor.transpose(fps[:], fT[:], identf[:])
                f_sb = acts.tile([128, 128], F32, tag=f"fsb{tag}")
                nc.scalar.activation(f_sb[:], fps[:], AF.Copy)
                nc.sync.dma_start(f_d[:, :], f_sb[:])
                pps = gps.tile([128, NCLS], F32, tag="gps")
                nc.tensor.transpose(pps[:], pT[:], identf[0:NCLS, 0:NCLS])
                p_sb = acts.tile([128, NCLS], F32, tag=f"psb{tag}")
                nc.scalar.activation(p_sb[:], pps[:], AF.Copy)
                nc.sync.dma_start(p_d[:, :], p_sb[:])

            if stage >= 6:
                emit_head(ovT, cvW1T, cvW2T, fv_d, pv_d, "v")
                emit_head(oaT, caW1T, caW2T, fa_d, pa_d, "a")

    nc.compile()
    return nc


def _prep_inputs(inputs):
    """Host-side: shard batch, transpose/cast weights. Returns list of in_maps."""
    bf = ml_dtypes.bfloat16
    f32 = np.float32

    def npa(x, dt=f32):
        return np.ascontiguousarray(np.asarray(x, dtype=f32)).astype(dt) \
            if dt is not f32 else np.ascontiguousarray(np.asarray(x, dtype=f32))

    img = npa(inputs["img"])          # [B, IMG]
    audio = npa(inputs["audio"])      # [B, AUD]
    vis_W = npa(inputs["vis_W"])      # [ENC, IMG]
    aud_W = npa(inputs["aud_W"])
    msv_W = npa(inputs["msv_W"])      # [S, D, ENC]
    msa_W = npa(inputs["msa_W"])
    mmfa_W = npa(inputs["mmfa_W"])    # [8, D, D]
    mmfa_b = npa(inputs["mmfa_b"])    # [8, D]
    bil_W = npa(inputs["bil_W"])      # [D, D, D] (o, i, j)
    out_W1 = npa(inputs["out_W1"])    # [OUT1, SD]
    out_W2 = npa(inputs["out_W2"])    # [LAT, OUT1]

    def bfc(x):
        return np.ascontiguousarray(x).astype(bf)

    shared = dict(
        visWT=bfc(vis_W.T),                       # [IMG, ENC]
        audWT=bfc(aud_W.T),
        msvWT=bfc(msv_W.reshape(SD, ENC).T),      # [ENC, SD]
        msaWT=bfc(msa_W.reshape(SD, ENC).T),
        mmfaWT=bfc(np.concatenate([mmfa_W[i].T for i in range(8)], axis=1)),
        bilWT=bfc(bil_W.transpose(1, 0, 2).reshape(D, D * D)),  # [i, (o,j)]
        outW1T=bfc(out_W1.T),
        outW2T=bfc(out_W2.T),
        cvW1T=bfc(npa(inputs["clsv_W1"]).T),
        cvW2T=bfc(npa(inputs["clsv_W2"]).T),
        caW1T=bfc(npa(inputs["clsa_W1"]).T),
        caW2T=bfc(npa(inputs["clsa_W2"]).T),
        visb_rep=np.tile(npa(inputs["vis_b"])[None, :], (128, 1)),
        audb_rep=np.tile(npa(inputs["aud_b"])[None, :], (128, 1)),
        msvb_rep=np.tile(npa(inputs["msv_b"]).reshape(-1)[None, :], (128, 1)),
        msab_rep=np.tile(npa(inputs["msa_b"]).reshape(-1)[None, :], (128, 1)),
        qkb_rep=np.tile(mmfa_b[0:4].reshape(-1)[None, :], (128, 1)),
        cb_cols=np.tile((mmfa_b[4] + mmfa_b[5])[:, None], (1, 1)).reshape(
            2, 128).T.copy(),  # placeholder, fixed below
        outb1T=npa(inputs["out_b1"]).reshape(4, 128).T.copy(),
        outb2T=npa(inputs["out_b2"]).reshape(128, 1).copy(),
        tc_rep=np.full((128, 1), float(np.asarray(inputs["t_c"])), f32),
    )
    # cb_cols: [128, 4], col ci*2+kc = chunk kc of (b4+b5) or (b6+b7)
    b45 = (mmfa_b[4] + mmfa_b[5]).astype(f32)
    b67 = (mmfa_b[6] + mmfa_b[7]).astype(f32)
    cb = np.zeros((128, 4), f32)
    cb[:, 0], cb[:, 1] = b45[0:128], b45[128:256]
    cb[:, 2], cb[:, 3] = b67[0:128], b67[128:256]
    shared["cb_cols"] = cb

    imgT = bfc(img.T)      # [IMG, B]
    audT = bfc(audio.T)
    in_maps = []
    for c in range(NCORES):
        m = dict(shared)
        m["imgT"] = np.ascontiguousarray(imgT[:, c * BL:(c + 1) * BL])
        m["audT"] = np.ascontiguousarray(audT[:, c * BL:(c + 1) * BL])
        in_maps.append(m)
    return in_maps


def kernel(**inputs):
    if "nc" not in _prog_cache:
        _prog_cache["nc"] = build_program()
    nc = _prog_cache["nc"]
    in_maps = _prep_inputs(inputs)
    res = run_bass_kernel_spmd(nc, in_maps, core_ids=list(range(NCORES)))
    outs = res.results
    fv = np.concatenate([outs[c]["fv"] for c in range(NCORES)], axis=0)
    fa = np.concatenate([outs[c]["fa"] for c in range(NCORES)], axis=0)
    pv = np.concatenate([outs[c]["pv"] for c in range(NCORES)], axis=0)
    pa = np.concatenate([outs[c]["pa"] for c in range(NCORES)], axis=0)
    return (fv.astype(np.float32), fa.astype(np.float32),
            pv.astype(np.float32), pa.astype(np.float32))
